# revision 2
# baseline (speedup 1.0000x reference)
"""Trainium2 Bass kernel for nn_Attention_9560597201123 — v4 head-parallel.

Sharding: 8 cores = (batch b) x (head half hh). Each core computes q/k/v
projections for its 8 heads over the full 2048-token sequence, attention,
and a PARTIAL output projection (contraction over its 512 dims). Host sums
the two head-half partials per batch and adds proj_b.

Schedule (all engines in-order; emission order is the schedule):
  - combined q|k|v weight tensor -> one full-rate DMA; biases first on sync
  - scores: two 64-row PE tiles run concurrently (row tiling)
  - PV lags scores by one kc so PE never waits on the exp latency
  - V chunks stream just ahead of the first unit's PVs
  - K/Q of the next pair and the out-projection are pumped as fillers
    (1-2 matmuls per kc) inside the attention units
  - ACT does exp only; DVE does all evictions (fused bias) and the softmax
    normalize via reciprocal_approx_fast
  - NO scalar-engine DMAs: they trigger a chip-wide ~1.2x downclock
"""

import numpy as np
import ml_dtypes

P = 128
SEQ = 2048
E = 1024
HPC = 8
NPAIR = 4
D = 64
KC = 16
EC = 8
QB = 4
SCALE = D ** -0.5
HDIM = 512

_NC = None


def build_nc():
    global _NC
    if _NC is not None:
        return _NC

    import concourse.bass as bass  # noqa: F401
    import concourse.mybir as mybir
    import concourse.tile as tile
    from concourse import bacc

    BF = mybir.dt.bfloat16
    F32 = mybir.dt.float32
    EXP = mybir.ActivationFunctionType.Exp
    ADD = mybir.AluOpType.add
    MULT = mybir.AluOpType.mult

    nc = bacc.Bacc("TRN2", target_bir_lowering=False, debug=False, num_devices=8)

    xt_d = nc.dram_tensor("xt", [E, SEQ], BF, kind="ExternalInput").ap()
    wqkv_d = nc.dram_tensor("wqkv", [E, 3 * HDIM], BF, kind="ExternalInput").ap()
    wp_d = nc.dram_tensor("wp", [HDIM, E], BF, kind="ExternalInput").ap()
    bq_d = nc.dram_tensor("bq", [HDIM], F32, kind="ExternalInput").ap()
    bk_d = nc.dram_tensor("bk", [HDIM], F32, kind="ExternalInput").ap()
    bv_bf_d = nc.dram_tensor("bv_bf", [HDIM], BF, kind="ExternalInput").ap()
    out_d = nc.dram_tensor("out", [SEQ, E], BF, kind="ExternalOutput").ap()

    xt_r = xt_d.rearrange("(o p) s -> p o s", p=P)        # [128, 8, 2048]
    wqkv_r = wqkv_d.rearrange("(o p) c -> p o c", p=P)    # [128, 8, 1536]
    wp_r = wp_d.rearrange("(o p) c -> p o c", p=P)        # [128, 4, 1024]

    with tile.TileContext(nc) as tc:
        with (
            tc.tile_pool(name="persist", bufs=1) as persist,
            tc.tile_pool(name="ptpool", bufs=8) as ptpool,
            tc.tile_pool(name="asbp", bufs=5) as asbp,
            tc.tile_pool(name="small", bufs=2) as small,
            tc.tile_pool(name="ypool", bufs=2) as ypool,
            tc.tile_pool(name="proj_ps", bufs=2, space="PSUM") as proj_ps,
            tc.tile_pool(name="acc_ps", bufs=2, space="PSUM") as acc_ps,
            tc.tile_pool(name="sc_ps", bufs=2, space="PSUM") as sc_ps,
        ):
            # ---- persistent tiles + input DMA ----
            # sync queue: biases, xt chunks 0,1 ; gpsimd: wqkv, xt 2,3, pw
            # single hardware-DGE queue (sync), transfers in need-order:
            # biases, q|k weights, xt chunks interleaved with v-weights, wp
            bq_t = persist.tile([P, NPAIR], F32, tag="bq_t")
            nc.sync.dma_start(bq_t[:], bq_d.rearrange("(o p) -> p o", p=P))
            bk_t = persist.tile([P, NPAIR], F32, tag="bk_t")
            nc.sync.dma_start(bk_t[:], bk_d.rearrange("(o p) -> p o", p=P))
            bv_row = persist.tile([1, HDIM], BF, tag="bv_row")
            nc.sync.dma_start(bv_row[:], bv_bf_d[None])

            xt3 = persist.tile([P, EC, SEQ], BF, tag="xt")
            wqkv_t = persist.tile([P, EC, 3 * HDIM], BF, tag="wqkv_t")
            nc.sync.dma_start(wqkv_t[:, :, 0 : 2 * HDIM], wqkv_r[:, :, 0 : 2 * HDIM])
            nc.sync.dma_start(xt3[:, :, 0:512], xt_r[:, :, 0:512])
            nc.sync.dma_start(xt3[:, :, 512:1024], xt_r[:, :, 512:1024])
            nc.sync.dma_start(wqkv_t[:, :, 2 * HDIM :], wqkv_r[:, :, 2 * HDIM :])
            nc.sync.dma_start(xt3[:, :, 1024:1536], xt_r[:, :, 1024:1536])
            nc.sync.dma_start(xt3[:, :, 1536:2048], xt_r[:, :, 1536:2048])
            pw = persist.tile([P, NPAIR, E], BF, tag="pw")
            nc.sync.dma_start(pw[:], wp_r[:])

            bv_bc = persist.tile([P, HDIM], BF, tag="bv_bc")
            nc.gpsimd.partition_broadcast(bv_bc[:], bv_row[:])

            vx = []
            for sm in range(KC):
                t = persist.tile([P, HPC * 65], BF, tag=f"vx{sm}", name=f"vx{sm}").rearrange(
                    "p (h c) -> p h c", c=65
                )
                nc.vector.memset(t[:, :, 64], 1.0)
                vx.append(t)

            kt = [persist.tile([P, SEQ], BF, tag=f"kt{p}", name=f"kt{p}") for p in range(NPAIR)]
            qt = [persist.tile([P, SEQ], BF, tag=f"qt{p}", name=f"qt{p}") for p in range(NPAIR)]
            aT = [persist.tile([P, SEQ], BF, tag=f"aT{p}", name=f"aT{p}") for p in range(NPAIR)]

            v_done = [0]
            k_parts = {}
            q_done = {}
            pending_norm = []
            flushed = []

            # ---- filler generators ----
            def gen_k(p, s):
                kcols = HDIM + p * P
                ssl = slice(s * 512, (s + 1) * 512)
                ps = proj_ps.tile([P, 512], F32, tag="ps512")
                for ec in range(EC):
                    nc.tensor.matmul(
                        ps[:],
                        lhsT=wqkv_t[:, ec, kcols : kcols + P],
                        rhs=xt3[:, ec, ssl],
                        start=(ec == 0),
                        stop=(ec == EC - 1),
                    )
                    yield
                nc.vector.tensor_scalar(
                    out=kt[p][:, ssl], in0=ps[:],
                    scalar1=bk_t[:, p : p + 1], scalar2=None, op0=ADD,
                )
                k_parts[p] = k_parts.get(p, 0) + 1

            def gen_q(p, qb):
                qcols = p * P
                ssl = slice(qb * 512, (qb + 1) * 512)
                ps = proj_ps.tile([P, 512], F32, tag="ps512")
                for ec in range(EC):
                    nc.tensor.matmul(
                        ps[:],
                        lhsT=wqkv_t[:, ec, qcols : qcols + P],
                        rhs=xt3[:, ec, ssl],
                        start=(ec == 0),
                        stop=(ec == EC - 1),
                    )
                    yield
                nc.vector.tensor_scalar(
                    out=qt[p][:, ssl], in0=ps[:],
                    scalar1=bq_t[:, p : p + 1], scalar2=None, op0=ADD,
                )
                q_done[(p, qb)] = True

            def gen_v(sm):
                ps = proj_ps.tile([P, 512], F32, tag="ps512")
                for ec in range(EC):
                    nc.tensor.matmul(
                        ps[:],
                        lhsT=xt3[:, ec, sm * P : (sm + 1) * P],
                        rhs=wqkv_t[:, ec, 2 * HDIM : 3 * HDIM],
                        start=(ec == 0),
                        stop=(ec == EC - 1),
                    )
                    yield
                nc.vector.tensor_tensor(
                    out=vx[sm][:, :, 0:64],
                    in0=ps[:].rearrange("p (h c) -> p h c", c=64),
                    in1=bv_bc[:].rearrange("p (h c) -> p h c", c=64),
                    op=ADD,
                )
                v_done[0] += 1

            def gen_outproj(qb):
                for j in range(4):
                    qc = qb * 4 + j
                    for ncol in range(2):
                        nsl = slice(ncol * 512, (ncol + 1) * 512)
                        yps = proj_ps.tile([P, 512], F32, tag="ps512")
                        for p in range(NPAIR):
                            nc.tensor.matmul(
                                yps[:],
                                lhsT=aT[p][:, qc * P : (qc + 1) * P],
                                rhs=pw[:, p, nsl],
                                start=(p == 0),
                                stop=(p == NPAIR - 1),
                            )
                            yield
                        ysb = ypool.tile([P, 512], BF, tag="ysb")
                        nc.vector.tensor_copy(out=ysb[:], in_=yps[:])
                        nc.sync.dma_start(out_d[qc * P : (qc + 1) * P, nsl], ysb[:])

            reg = []

            def pump_reg(n):
                done = 0
                while reg and done < n:
                    try:
                        next(reg[0])
                        done += 1
                    except StopIteration:
                        reg.pop(0)
                return done

            def flush_norm():
                for key, fn in pending_norm:
                    fn()
                    flushed.append(key)
                pending_norm.clear()

            def emit_unit(p, qb, rate=2, v_follow=False):
                """Attention unit. Inline PVs lag scores by one kc. When
                v_follow, V-proj chunks are emitted just ahead of the PVs."""
                need_k = 1 if p == 0 else 4
                while not (k_parts.get(p, 0) >= need_k and q_done.get((p, qb))):
                    if not pump_reg(8):
                        raise RuntimeError(f"missing k/q fillers for {(p, qb)}")
                qsl = slice(qb * 512, (qb + 1) * 512)
                accA = acc_ps.tile([65, 512], F32, tag="acc")
                accB = acc_ps.tile([65, 512], F32, tag="acc")
                pts = [None] * KC

                def pv(kc):
                    nc.tensor.matmul(
                        accA[:],
                        lhsT=vx[kc][:, 2 * p, :],
                        rhs=pts[kc][:, 0:512],
                        start=(kc == 0),
                        stop=(kc == KC - 1),
                    )
                    nc.tensor.matmul(
                        accB[:],
                        lhsT=vx[kc][:, 2 * p + 1, :],
                        rhs=pts[kc][:, 512:1024],
                        start=(kc == 0),
                        stop=(kc == KC - 1),
                    )

                for kc in range(KC):
                    while k_parts.get(p, 0) <= kc // 4:
                        if not pump_reg(8):
                            raise RuntimeError(f"missing kt chunk for {(p, kc)}")
                    sc = sc_ps.tile([P, 1024], F32, tag="sc")
                    nc.tensor.matmul(
                        sc[:, 0:512],
                        lhsT=kt[p][0:64, kc * P : (kc + 1) * P],
                        rhs=qt[p][0:64, qsl],
                    )
                    nc.tensor.matmul(
                        sc[:, 512:1024],
                        lhsT=kt[p][64:P, kc * P : (kc + 1) * P],
                        rhs=qt[p][64:P, qsl],
                    )
                    pt = ptpool.tile([P, 1024], BF, tag="pt")
                    nc.scalar.activation(out=pt[:], in_=sc[:], func=EXP, scale=SCALE)
                    pts[kc] = pt
                    if v_follow:
                        # keep V emission >= 1 chunk ahead of the lagged PV
                        while v_done[0] <= min(kc + 1, KC - 1):
                            if not pump_reg(8):
                                break
                    else:
                        pump_reg(rate)
                    if kc > 0:
                        pv(kc - 1)
                pv(KC - 1)

                key = (p, qb)
                for hh, acc in ((0, accA), (1, accB)):
                    asb = asbp.tile([65, 512], F32, tag="asb")
                    nc.vector.tensor_copy(out=asb[:], in_=acc[:])

                    def norm(asb=asb, hh=hh, p=p, qsl=qsl):
                        s0 = small.tile([1, 512], F32, tag="s0")
                        nc.vector.tensor_copy(out=s0[:], in_=asb[64:65, :])
                        rs = small.tile([1, 512], F32, tag="rs")
                        nc.vector.reciprocal_approx_fast(rs[:], s0[:])
                        R = small.tile([64, 512], F32, tag="R")
                        nc.gpsimd.partition_broadcast(R[:], rs[:])
                        if hh == 0:
                            nc.vector.tensor_tensor(
                                out=aT[p][0:64, qsl], in0=asb[0:64, :], in1=R[:],
                                op=MULT,
                            )
                        else:
                            tmpb = small.tile([64, 512], BF, tag="tmpb")
                            nc.vector.tensor_tensor(
                                out=tmpb[:], in0=asb[0:64, :], in1=R[:], op=MULT
                            )
                            nc.sync.dma_start(aT[p][64:P, qsl], tmpb[:])

                    pending_norm.append((key, norm))

            # ---- schedule ----
            # dense pre-attention: K0 seq chunks 0-1 + Q0 qb0; the rest
            # stream as fillers in dependency order
            for s in range(2):
                for _ in gen_k(0, s):
                    pass
            for _ in gen_q(0, 0):
                pass
            reg.append(gen_k(0, 2))
            reg.append(gen_k(0, 3))
            reg.append(gen_q(0, 1))
            for sm in range(KC):
                reg.append(gen_v(sm))
            reg.append(gen_q(0, 2))
            reg.append(gen_q(0, 3))

            for p in range(NPAIR):
                for qb in range(QB):
                    emit_unit(p, qb, v_follow=(p == 0 and qb <= 1))
                    flush_norm()
                    for key in dict.fromkeys(flushed):
                        if key[0] == NPAIR - 1:
                            reg.append(gen_outproj(key[1]))
                    flushed.clear()
                    if p + 1 < NPAIR:
                        if qb == 0:
                            reg.append(gen_k(p + 1, 0))
                            reg.append(gen_k(p + 1, 1))
                        elif qb == 1:
                            reg.append(gen_k(p + 1, 2))
                            reg.append(gen_k(p + 1, 3))
                            reg.append(gen_q(p + 1, 0))
                        elif qb == 2:
                            reg.append(gen_q(p + 1, 1))
                            reg.append(gen_q(p + 1, 2))
                            reg.append(gen_q(p + 1, 3))

            # epilogue
            flush_norm()
            for key in dict.fromkeys(flushed):
                if key[0] == NPAIR - 1:
                    reg.append(gen_outproj(key[1]))
            flushed.clear()
            while reg:
                pump_reg(1 << 30)

    nc.finalize()
    _NC = nc
    return nc


def make_in_maps(x, qkv_w, qkv_b, proj_w, proj_b):
    bf16 = ml_dtypes.bfloat16
    x = np.asarray(x, dtype=np.float32)
    qkv_w = np.asarray(qkv_w, dtype=np.float32)
    qkv_b = np.asarray(qkv_b, dtype=np.float32)
    proj_w = np.asarray(proj_w, dtype=np.float32)
    in_maps = []
    for c in range(8):
        b, hh = divmod(c, 2)
        qsl = slice(hh * HDIM, (hh + 1) * HDIM)
        ksl = slice(E + hh * HDIM, E + (hh + 1) * HDIM)
        vsl = slice(2 * E + hh * HDIM, 2 * E + (hh + 1) * HDIM)
        wqkv = np.concatenate([qkv_w[:, qsl], qkv_w[:, ksl], qkv_w[:, vsl]], axis=1)
        in_maps.append(
            {
                "xt": np.ascontiguousarray(x[b].T).astype(bf16),
                "wqkv": np.ascontiguousarray(wqkv).astype(bf16),
                "wp": np.ascontiguousarray(proj_w[qsl, :]).astype(bf16),
                "bq": np.ascontiguousarray(qkv_b[qsl]),
                "bk": np.ascontiguousarray(qkv_b[ksl]),
                "bv_bf": np.ascontiguousarray(qkv_b[vsl]).astype(bf16),
            }
        )
    return in_maps


def assemble_out(results, proj_b):
    out = np.empty((4, SEQ, E), dtype=np.float32)
    pb = np.asarray(proj_b, dtype=np.float32)
    for b in range(4):
        a = np.asarray(results[2 * b]["out"], dtype=np.float32)
        c = np.asarray(results[2 * b + 1]["out"], dtype=np.float32)
        out[b] = a + c + pb
    return out


def run(inputs, trace=False):
    from concourse.bass_utils import run_bass_kernel_spmd

    nc = build_nc()
    in_maps = make_in_maps(**inputs)
    res = run_bass_kernel_spmd(nc, in_maps, core_ids=list(range(8)), trace=trace)
    return assemble_out(res.results, inputs["proj_b"]), res


def kernel(x, qkv_w, qkv_b, proj_w, proj_b):
    out, _ = run(
        dict(x=x, qkv_w=qkv_w, qkv_b=qkv_b, proj_w=proj_w, proj_b=proj_b),
        trace=False,
    )
    return out


# revision 3
# speedup vs baseline: 1.0084x; 1.0084x over previous
"""Trainium2 Bass kernel for nn_Attention_9560597201123 — v4 head-parallel.

Sharding: 8 cores = (batch b) x (head half hh). Each core computes q/k/v
projections for its 8 heads over the full 2048-token sequence, attention,
and a PARTIAL output projection (contraction over its 512 dims). Host sums
the two head-half partials per batch and adds proj_b.

Schedule (all engines in-order; emission order is the schedule):
  - combined q|k|v weight tensor -> one full-rate DMA; biases first on sync
  - scores: two 64-row PE tiles run concurrently (row tiling)
  - PV lags scores by one kc so PE never waits on the exp latency
  - V chunks stream just ahead of the first unit's PVs
  - K/Q of the next pair and the out-projection are pumped as fillers
    (1-2 matmuls per kc) inside the attention units
  - ACT does exp only; DVE does all evictions (fused bias) and the softmax
    normalize via reciprocal_approx_fast
  - NO scalar-engine DMAs: they trigger a chip-wide ~1.2x downclock
"""

import numpy as np
import ml_dtypes

P = 128
SEQ = 2048
E = 1024
HPC = 8
NPAIR = 4
D = 64
KC = 16
EC = 8
QB = 4
SCALE = D ** -0.5
HDIM = 512

_NC = None


def build_nc():
    global _NC
    if _NC is not None:
        return _NC

    import concourse.bass as bass  # noqa: F401
    import concourse.mybir as mybir
    import concourse.tile as tile
    from concourse import bacc

    BF = mybir.dt.bfloat16
    F32 = mybir.dt.float32
    EXP = mybir.ActivationFunctionType.Exp
    ADD = mybir.AluOpType.add
    MULT = mybir.AluOpType.mult

    nc = bacc.Bacc("TRN2", target_bir_lowering=False, debug=False, num_devices=8)

    xt_d = nc.dram_tensor("xt", [E, SEQ], BF, kind="ExternalInput").ap()
    wqkv_d = nc.dram_tensor("wqkv", [E, 3 * HDIM], BF, kind="ExternalInput").ap()
    wp_d = nc.dram_tensor("wp", [HDIM, E], BF, kind="ExternalInput").ap()
    bq_d = nc.dram_tensor("bq", [HDIM], F32, kind="ExternalInput").ap()
    bk_d = nc.dram_tensor("bk", [HDIM], F32, kind="ExternalInput").ap()
    bv_bf_d = nc.dram_tensor("bv_bf", [HDIM], BF, kind="ExternalInput").ap()
    out_d = nc.dram_tensor("out", [SEQ, E], BF, kind="ExternalOutput").ap()

    xt_r = xt_d.rearrange("(o p) s -> p o s", p=P)        # [128, 8, 2048]
    wqkv_r = wqkv_d.rearrange("(o p) c -> p o c", p=P)    # [128, 8, 1536]
    wp_r = wp_d.rearrange("(o p) c -> p o c", p=P)        # [128, 4, 1024]

    with tile.TileContext(nc) as tc:
        with (
            tc.tile_pool(name="persist", bufs=1) as persist,
            tc.tile_pool(name="ptpool", bufs=8) as ptpool,
            tc.tile_pool(name="asbp", bufs=5) as asbp,
            tc.tile_pool(name="small", bufs=2) as small,
            tc.tile_pool(name="ypool", bufs=2) as ypool,
            tc.tile_pool(name="proj_ps", bufs=2, space="PSUM") as proj_ps,
            tc.tile_pool(name="acc_ps", bufs=2, space="PSUM") as acc_ps,
            tc.tile_pool(name="sc_ps", bufs=2, space="PSUM") as sc_ps,
        ):
            # ---- persistent tiles + input DMA ----
            # sync queue: biases, xt chunks 0,1 ; gpsimd: wqkv, xt 2,3, pw
            # single hardware-DGE queue (sync), transfers in need-order:
            # biases, q|k weights, xt chunks interleaved with v-weights, wp
            bq_t = persist.tile([P, NPAIR], F32, tag="bq_t")
            nc.sync.dma_start(bq_t[:], bq_d.rearrange("(o p) -> p o", p=P))
            bk_t = persist.tile([P, NPAIR], F32, tag="bk_t")
            nc.sync.dma_start(bk_t[:], bk_d.rearrange("(o p) -> p o", p=P))
            bv_row = persist.tile([1, HDIM], BF, tag="bv_row")
            nc.sync.dma_start(bv_row[:], bv_bf_d[None])

            # xt in 1024-col chunks: 2KB DMA lines run at full rate
            xt3 = persist.tile([P, EC, SEQ], BF, tag="xt")
            wqkv_t = persist.tile([P, EC, 3 * HDIM], BF, tag="wqkv_t")
            nc.sync.dma_start(wqkv_t[:, :, 0:HDIM], wqkv_r[:, :, 0:HDIM])
            nc.sync.dma_start(xt3[:, :, 0:1024], xt_r[:, :, 0:1024])
            nc.sync.dma_start(wqkv_t[:, :, HDIM : 2 * HDIM], wqkv_r[:, :, HDIM : 2 * HDIM])
            nc.sync.dma_start(wqkv_t[:, :, 2 * HDIM :], wqkv_r[:, :, 2 * HDIM :])
            nc.sync.dma_start(xt3[:, :, 1024:2048], xt_r[:, :, 1024:2048])
            pw = persist.tile([P, NPAIR, E], BF, tag="pw")
            nc.sync.dma_start(pw[:], wp_r[:])

            bv_bc = persist.tile([P, HDIM], BF, tag="bv_bc")
            nc.gpsimd.partition_broadcast(bv_bc[:], bv_row[:])

            vx = []
            for sm in range(KC):
                t = persist.tile([P, HPC * 65], BF, tag=f"vx{sm}", name=f"vx{sm}").rearrange(
                    "p (h c) -> p h c", c=65
                )
                nc.vector.memset(t[:, :, 64], 1.0)
                vx.append(t)

            kt = [persist.tile([P, SEQ], BF, tag=f"kt{p}", name=f"kt{p}") for p in range(NPAIR)]
            qt = [persist.tile([P, SEQ], BF, tag=f"qt{p}", name=f"qt{p}") for p in range(NPAIR)]
            aT = [persist.tile([P, SEQ], BF, tag=f"aT{p}", name=f"aT{p}") for p in range(NPAIR)]

            v_done = [0]
            k_parts = {}
            q_done = {}
            pending_norm = []
            flushed = []

            # ---- filler generators ----
            def gen_k(p, s):
                kcols = p * P
                ssl = slice(s * 512, (s + 1) * 512)
                ps = proj_ps.tile([P, 512], F32, tag="ps512")
                for ec in range(EC):
                    nc.tensor.matmul(
                        ps[:],
                        lhsT=wqkv_t[:, ec, kcols : kcols + P],
                        rhs=xt3[:, ec, ssl],
                        start=(ec == 0),
                        stop=(ec == EC - 1),
                    )
                    yield
                nc.vector.tensor_scalar(
                    out=kt[p][:, ssl], in0=ps[:],
                    scalar1=bk_t[:, p : p + 1], scalar2=None, op0=ADD,
                )
                k_parts[p] = k_parts.get(p, 0) + 1

            def gen_q(p, qb):
                qcols = HDIM + p * P
                ssl = slice(qb * 512, (qb + 1) * 512)
                ps = proj_ps.tile([P, 512], F32, tag="ps512")
                for ec in range(EC):
                    nc.tensor.matmul(
                        ps[:],
                        lhsT=wqkv_t[:, ec, qcols : qcols + P],
                        rhs=xt3[:, ec, ssl],
                        start=(ec == 0),
                        stop=(ec == EC - 1),
                    )
                    yield
                nc.vector.tensor_scalar(
                    out=qt[p][:, ssl], in0=ps[:],
                    scalar1=bq_t[:, p : p + 1], scalar2=None, op0=ADD,
                )
                q_done[(p, qb)] = True

            def gen_v(sm):
                ps = proj_ps.tile([P, 512], F32, tag="ps512")
                for ec in range(EC):
                    nc.tensor.matmul(
                        ps[:],
                        lhsT=xt3[:, ec, sm * P : (sm + 1) * P],
                        rhs=wqkv_t[:, ec, 2 * HDIM : 3 * HDIM],
                        start=(ec == 0),
                        stop=(ec == EC - 1),
                    )
                    yield
                nc.vector.tensor_tensor(
                    out=vx[sm][:, :, 0:64],
                    in0=ps[:].rearrange("p (h c) -> p h c", c=64),
                    in1=bv_bc[:].rearrange("p (h c) -> p h c", c=64),
                    op=ADD,
                )
                v_done[0] += 1

            def gen_outproj(qb):
                for j in range(4):
                    qc = qb * 4 + j
                    for ncol in range(2):
                        nsl = slice(ncol * 512, (ncol + 1) * 512)
                        yps = proj_ps.tile([P, 512], F32, tag="ps512")
                        for p in range(NPAIR):
                            nc.tensor.matmul(
                                yps[:],
                                lhsT=aT[p][:, qc * P : (qc + 1) * P],
                                rhs=pw[:, p, nsl],
                                start=(p == 0),
                                stop=(p == NPAIR - 1),
                            )
                            yield
                        ysb = ypool.tile([P, 512], BF, tag="ysb")
                        nc.vector.tensor_copy(out=ysb[:], in_=yps[:])
                        nc.sync.dma_start(out_d[qc * P : (qc + 1) * P, nsl], ysb[:])

            reg = []
            vq = []

            def pump_vq():
                while vq:
                    try:
                        next(vq[0])
                        return True
                    except StopIteration:
                        vq.pop(0)
                return False

            def pump_reg(n):
                done = 0
                while reg and done < n:
                    try:
                        next(reg[0])
                        done += 1
                    except StopIteration:
                        reg.pop(0)
                return done

            def flush_norm():
                for key, fn in pending_norm:
                    fn()
                    flushed.append(key)
                pending_norm.clear()

            def emit_unit(p, qb, rate=2, v_follow=False):
                """Attention unit. Inline PVs lag scores by one kc. When
                v_follow, V-proj chunks are emitted just ahead of the PVs."""
                need_k = 1 if p == 0 else 4
                while not (k_parts.get(p, 0) >= need_k and q_done.get((p, qb))):
                    if not pump_reg(8):
                        raise RuntimeError(f"missing k/q fillers for {(p, qb)}")
                qsl = slice(qb * 512, (qb + 1) * 512)
                accA = acc_ps.tile([65, 512], F32, tag="acc")
                accB = acc_ps.tile([65, 512], F32, tag="acc")
                pts = [None] * KC

                def pv(kc):
                    nc.tensor.matmul(
                        accA[:],
                        lhsT=vx[kc][:, 2 * p, :],
                        rhs=pts[kc][:, 0:512],
                        start=(kc == 0),
                        stop=(kc == KC - 1),
                    )
                    nc.tensor.matmul(
                        accB[:],
                        lhsT=vx[kc][:, 2 * p + 1, :],
                        rhs=pts[kc][:, 512:1024],
                        start=(kc == 0),
                        stop=(kc == KC - 1),
                    )

                for kc in range(KC):
                    while k_parts.get(p, 0) <= kc // 4:
                        if not pump_reg(8):
                            raise RuntimeError(f"missing kt chunk for {(p, kc)}")
                    sc = sc_ps.tile([P, 1024], F32, tag="sc")
                    nc.tensor.matmul(
                        sc[:, 0:512],
                        lhsT=kt[p][0:64, kc * P : (kc + 1) * P],
                        rhs=qt[p][0:64, qsl],
                    )
                    nc.tensor.matmul(
                        sc[:, 512:1024],
                        lhsT=kt[p][64:P, kc * P : (kc + 1) * P],
                        rhs=qt[p][64:P, qsl],
                    )
                    pt = ptpool.tile([P, 1024], BF, tag="pt")
                    nc.scalar.activation(out=pt[:], in_=sc[:], func=EXP, scale=SCALE)
                    pts[kc] = pt
                    if v_follow:
                        # keep V emission >= 1 chunk ahead of the lagged PV
                        while v_done[0] <= min(kc + 1, KC - 1):
                            if not pump_vq():
                                break
                        pump_reg(1)
                    else:
                        while vq:
                            pump_vq()
                        pump_reg(rate)
                    if kc > 0:
                        pv(kc - 1)
                pv(KC - 1)

                key = (p, qb)
                for hh, acc in ((0, accA), (1, accB)):
                    asb = asbp.tile([65, 512], F32, tag="asb")
                    nc.vector.tensor_copy(out=asb[:], in_=acc[:])

                    def norm(asb=asb, hh=hh, p=p, qsl=qsl):
                        s0 = small.tile([1, 512], F32, tag="s0")
                        nc.vector.tensor_copy(out=s0[:], in_=asb[64:65, :])
                        rs = small.tile([1, 512], F32, tag="rs")
                        nc.vector.reciprocal_approx_fast(rs[:], s0[:])
                        R = small.tile([64, 512], F32, tag="R")
                        nc.gpsimd.partition_broadcast(R[:], rs[:])
                        if hh == 0:
                            nc.vector.tensor_tensor(
                                out=aT[p][0:64, qsl], in0=asb[0:64, :], in1=R[:],
                                op=MULT,
                            )
                        else:
                            tmpb = small.tile([64, 512], BF, tag="tmpb")
                            nc.vector.tensor_tensor(
                                out=tmpb[:], in0=asb[0:64, :], in1=R[:], op=MULT
                            )
                            nc.sync.dma_start(aT[p][64:P, qsl], tmpb[:])

                    pending_norm.append((key, norm))

            # ---- schedule ----
            # dense pre-attention: K0 seq chunk 0 + Q0 qb0; the rest
            # streams as fillers in dependency order
            for _ in gen_k(0, 0):
                pass
            for _ in gen_q(0, 0):
                pass
            reg.append(gen_k(0, 1))
            reg.append(gen_k(0, 2))
            reg.append(gen_k(0, 3))
            reg.append(gen_q(0, 1))
            for sm in range(KC):
                vq.append(gen_v(sm))
            reg.append(gen_q(0, 2))
            reg.append(gen_q(0, 3))

            for p in range(NPAIR):
                for qb in range(QB):
                    emit_unit(p, qb, v_follow=(p == 0 and qb <= 1))
                    flush_norm()
                    for key in dict.fromkeys(flushed):
                        if key[0] == NPAIR - 1:
                            reg.append(gen_outproj(key[1]))
                    flushed.clear()
                    if p + 1 < NPAIR:
                        if qb == 0:
                            reg.append(gen_k(p + 1, 0))
                            reg.append(gen_k(p + 1, 1))
                        elif qb == 1:
                            reg.append(gen_k(p + 1, 2))
                            reg.append(gen_k(p + 1, 3))
                            reg.append(gen_q(p + 1, 0))
                        elif qb == 2:
                            reg.append(gen_q(p + 1, 1))
                            reg.append(gen_q(p + 1, 2))
                            reg.append(gen_q(p + 1, 3))

            # epilogue
            flush_norm()
            for key in dict.fromkeys(flushed):
                if key[0] == NPAIR - 1:
                    reg.append(gen_outproj(key[1]))
            flushed.clear()
            while reg:
                pump_reg(1 << 30)

    nc.finalize()
    _NC = nc
    return nc


def make_in_maps(x, qkv_w, qkv_b, proj_w, proj_b):
    bf16 = ml_dtypes.bfloat16
    x = np.asarray(x, dtype=np.float32)
    qkv_w = np.asarray(qkv_w, dtype=np.float32)
    qkv_b = np.asarray(qkv_b, dtype=np.float32)
    proj_w = np.asarray(proj_w, dtype=np.float32)
    in_maps = []
    for c in range(8):
        b, hh = divmod(c, 2)
        qsl = slice(hh * HDIM, (hh + 1) * HDIM)
        ksl = slice(E + hh * HDIM, E + (hh + 1) * HDIM)
        vsl = slice(2 * E + hh * HDIM, 2 * E + (hh + 1) * HDIM)
        wqkv = np.concatenate([qkv_w[:, ksl], qkv_w[:, qsl], qkv_w[:, vsl]], axis=1)
        in_maps.append(
            {
                "xt": np.ascontiguousarray(x[b].T).astype(bf16),
                "wqkv": np.ascontiguousarray(wqkv).astype(bf16),
                "wp": np.ascontiguousarray(proj_w[qsl, :]).astype(bf16),
                "bq": np.ascontiguousarray(qkv_b[qsl]),
                "bk": np.ascontiguousarray(qkv_b[ksl]),
                "bv_bf": np.ascontiguousarray(qkv_b[vsl]).astype(bf16),
            }
        )
    return in_maps


def assemble_out(results, proj_b):
    out = np.empty((4, SEQ, E), dtype=np.float32)
    pb = np.asarray(proj_b, dtype=np.float32)
    for b in range(4):
        a = np.asarray(results[2 * b]["out"], dtype=np.float32)
        c = np.asarray(results[2 * b + 1]["out"], dtype=np.float32)
        out[b] = a + c + pb
    return out


def run(inputs, trace=False):
    from concourse.bass_utils import run_bass_kernel_spmd

    nc = build_nc()
    in_maps = make_in_maps(**inputs)
    res = run_bass_kernel_spmd(nc, in_maps, core_ids=list(range(8)), trace=trace)
    return assemble_out(res.results, inputs["proj_b"]), res


def kernel(x, qkv_w, qkv_b, proj_w, proj_b):
    out, _ = run(
        dict(x=x, qkv_w=qkv_w, qkv_b=qkv_b, proj_w=proj_w, proj_b=proj_b),
        trace=False,
    )
    return out


# revision 4
# speedup vs baseline: 1.0200x; 1.0115x over previous
"""Trainium2 Bass kernel for nn_Attention_9560597201123 — v4 head-parallel.

Sharding: 8 cores = (batch b) x (head half hh). Each core computes q/k/v
projections for its 8 heads over the full 2048-token sequence, attention,
and a PARTIAL output projection (contraction over its 512 dims). Host sums
the two head-half partials per batch and adds proj_b.

Schedule (all engines in-order; emission order is the schedule):
  - combined q|k|v weight tensor -> one full-rate DMA; biases first on sync
  - scores: two 64-row PE tiles run concurrently (row tiling)
  - PV lags scores by one kc so PE never waits on the exp latency
  - V chunks stream just ahead of the first unit's PVs
  - K/Q of the next pair and the out-projection are pumped as fillers
    (1-2 matmuls per kc) inside the attention units
  - ACT does exp only; DVE does all evictions (fused bias) and the softmax
    normalize via reciprocal_approx_fast
  - NO scalar-engine DMAs: they trigger a chip-wide ~1.2x downclock
"""

import numpy as np
import ml_dtypes

P = 128
SEQ = 2048
E = 1024
HPC = 8
NPAIR = 4
D = 64
KC = 16
EC = 8
QB = 4
SCALE = D ** -0.5
HDIM = 512

_NC = None


def build_nc():
    global _NC
    if _NC is not None:
        return _NC

    import concourse.bass as bass  # noqa: F401
    import concourse.mybir as mybir
    import concourse.tile as tile
    from concourse import bacc

    BF = mybir.dt.bfloat16
    F32 = mybir.dt.float32
    EXP = mybir.ActivationFunctionType.Exp
    ADD = mybir.AluOpType.add
    MULT = mybir.AluOpType.mult

    nc = bacc.Bacc("TRN2", target_bir_lowering=False, debug=False, num_devices=8)

    xt_d = nc.dram_tensor("xt", [E, SEQ], BF, kind="ExternalInput").ap()
    wqkv_d = nc.dram_tensor("wqkv", [E, 3 * HDIM], BF, kind="ExternalInput").ap()
    wp_d = nc.dram_tensor("wp", [HDIM, E], BF, kind="ExternalInput").ap()
    bq_d = nc.dram_tensor("bq", [HDIM], F32, kind="ExternalInput").ap()
    bk_d = nc.dram_tensor("bk", [HDIM], F32, kind="ExternalInput").ap()
    bv_bf_d = nc.dram_tensor("bv_bf", [HDIM], BF, kind="ExternalInput").ap()
    out_d = nc.dram_tensor("out", [SEQ, E], BF, kind="ExternalOutput").ap()

    xt_r = xt_d.rearrange("(o p) s -> p o s", p=P)        # [128, 8, 2048]
    wqkv_r = wqkv_d.rearrange("(o p) c -> p o c", p=P)    # [128, 8, 1536]
    wp_r = wp_d.rearrange("(o p) c -> p o c", p=P)        # [128, 4, 1024]

    with tile.TileContext(nc) as tc:
        with (
            tc.tile_pool(name="persist", bufs=1) as persist,
            tc.tile_pool(name="ptpool", bufs=20) as ptpool,
            tc.tile_pool(name="asbp", bufs=5) as asbp,
            tc.tile_pool(name="small", bufs=2) as small,
            tc.tile_pool(name="ypool", bufs=2) as ypool,
            tc.tile_pool(name="proj_ps", bufs=2, space="PSUM") as proj_ps,
            tc.tile_pool(name="acc_ps", bufs=2, space="PSUM") as acc_ps,
            tc.tile_pool(name="sc_ps", bufs=2, space="PSUM") as sc_ps,
        ):
            # ---- persistent tiles + input DMA ----
            # sync queue: biases, xt chunks 0,1 ; gpsimd: wqkv, xt 2,3, pw
            # single hardware-DGE queue (sync), transfers in need-order:
            # biases, q|k weights, xt chunks interleaved with v-weights, wp
            bq_t = persist.tile([P, NPAIR], F32, tag="bq_t")
            nc.sync.dma_start(bq_t[:], bq_d.rearrange("(o p) -> p o", p=P))
            bk_t = persist.tile([P, NPAIR], F32, tag="bk_t")
            nc.sync.dma_start(bk_t[:], bk_d.rearrange("(o p) -> p o", p=P))
            bv_row = persist.tile([1, HDIM], BF, tag="bv_row")
            nc.sync.dma_start(bv_row[:], bv_bf_d[None])

            # xt in 1024-col chunks: 2KB DMA lines run at full rate
            xt3 = persist.tile([P, EC, SEQ], BF, tag="xt")
            wqkv_t = persist.tile([P, EC, 3 * HDIM], BF, tag="wqkv_t")
            nc.sync.dma_start(wqkv_t[:, :, 0:HDIM], wqkv_r[:, :, 0:HDIM])
            nc.sync.dma_start(xt3[:, :, 0:1024], xt_r[:, :, 0:1024])
            nc.sync.dma_start(wqkv_t[:, :, HDIM : 2 * HDIM], wqkv_r[:, :, HDIM : 2 * HDIM])
            nc.sync.dma_start(wqkv_t[:, :, 2 * HDIM :], wqkv_r[:, :, 2 * HDIM :])
            nc.sync.dma_start(xt3[:, :, 1024:2048], xt_r[:, :, 1024:2048])
            pw = persist.tile([P, NPAIR, E], BF, tag="pw")
            nc.sync.dma_start(pw[:], wp_r[:])

            bv_bc = persist.tile([P, HDIM], BF, tag="bv_bc")
            nc.gpsimd.partition_broadcast(bv_bc[:], bv_row[:])

            vx = []
            for sm in range(KC):
                t = persist.tile([P, HPC * 65], BF, tag=f"vx{sm}", name=f"vx{sm}").rearrange(
                    "p (h c) -> p h c", c=65
                )
                nc.vector.memset(t[:, :, 64], 1.0)
                vx.append(t)

            kt = [persist.tile([P, SEQ], BF, tag=f"kt{p}", name=f"kt{p}") for p in range(NPAIR)]
            qt = [persist.tile([P, SEQ], BF, tag=f"qt{p}", name=f"qt{p}") for p in range(NPAIR)]
            aT = [persist.tile([P, SEQ], BF, tag=f"aT{p}", name=f"aT{p}") for p in range(NPAIR)]

            v_done = [0]
            k_parts = {}
            q_done = {}
            pending_norm = []
            flushed = []

            # ---- filler generators ----
            def gen_k(p, s):
                kcols = p * P
                ssl = slice(s * 512, (s + 1) * 512)
                ps = proj_ps.tile([P, 512], F32, tag="ps512")
                for ec in range(EC):
                    nc.tensor.matmul(
                        ps[:],
                        lhsT=wqkv_t[:, ec, kcols : kcols + P],
                        rhs=xt3[:, ec, ssl],
                        start=(ec == 0),
                        stop=(ec == EC - 1),
                    )
                    yield
                nc.vector.tensor_scalar(
                    out=kt[p][:, ssl], in0=ps[:],
                    scalar1=bk_t[:, p : p + 1], scalar2=None, op0=ADD,
                )
                k_parts[p] = k_parts.get(p, 0) + 1

            def gen_q(p, qb):
                qcols = HDIM + p * P
                ssl = slice(qb * 512, (qb + 1) * 512)
                ps = proj_ps.tile([P, 512], F32, tag="ps512")
                for ec in range(EC):
                    nc.tensor.matmul(
                        ps[:],
                        lhsT=wqkv_t[:, ec, qcols : qcols + P],
                        rhs=xt3[:, ec, ssl],
                        start=(ec == 0),
                        stop=(ec == EC - 1),
                    )
                    yield
                nc.vector.tensor_scalar(
                    out=qt[p][:, ssl], in0=ps[:],
                    scalar1=bq_t[:, p : p + 1], scalar2=None, op0=ADD,
                )
                q_done[(p, qb)] = True

            def gen_v(sm):
                ps = proj_ps.tile([P, 512], F32, tag="ps512")
                for ec in range(EC):
                    nc.tensor.matmul(
                        ps[:],
                        lhsT=xt3[:, ec, sm * P : (sm + 1) * P],
                        rhs=wqkv_t[:, ec, 2 * HDIM : 3 * HDIM],
                        start=(ec == 0),
                        stop=(ec == EC - 1),
                    )
                    yield
                nc.vector.tensor_tensor(
                    out=vx[sm][:, :, 0:64],
                    in0=ps[:].rearrange("p (h c) -> p h c", c=64),
                    in1=bv_bc[:].rearrange("p (h c) -> p h c", c=64),
                    op=ADD,
                )
                v_done[0] += 1

            def gen_outproj(qb):
                for j in range(4):
                    qc = qb * 4 + j
                    for ncol in range(2):
                        nsl = slice(ncol * 512, (ncol + 1) * 512)
                        yps = proj_ps.tile([P, 512], F32, tag="ps512")
                        for p in range(NPAIR):
                            nc.tensor.matmul(
                                yps[:],
                                lhsT=aT[p][:, qc * P : (qc + 1) * P],
                                rhs=pw[:, p, nsl],
                                start=(p == 0),
                                stop=(p == NPAIR - 1),
                            )
                            yield
                        ysb = ypool.tile([P, 512], BF, tag="ysb")
                        nc.vector.tensor_copy(out=ysb[:], in_=yps[:])
                        nc.sync.dma_start(out_d[qc * P : (qc + 1) * P, nsl], ysb[:])

            reg = []
            vq = []

            def pump_vq():
                while vq:
                    try:
                        next(vq[0])
                        return True
                    except StopIteration:
                        vq.pop(0)
                return False

            def pump_reg(n):
                done = 0
                while reg and done < n:
                    try:
                        next(reg[0])
                        done += 1
                    except StopIteration:
                        reg.pop(0)
                return done

            def flush_norm():
                for key, fn in pending_norm:
                    fn()
                    flushed.append(key)
                pending_norm.clear()

            pv_gen = [None]

            def gen_unit_pv(p, qb, pts):
                """Deferred PV + evict for a finished unit, run in lockstep
                with the next unit's kc loop (caller guarantees V emission)."""
                accA = acc_ps.tile([65, 512], F32, tag="acc")
                accB = acc_ps.tile([65, 512], F32, tag="acc")
                for kc in range(KC):
                    assert v_done[0] > kc
                    nc.tensor.matmul(
                        accA[:],
                        lhsT=vx[kc][:, 2 * p, :],
                        rhs=pts[kc][:, 0:512],
                        start=(kc == 0),
                        stop=(kc == KC - 1),
                    )
                    nc.tensor.matmul(
                        accB[:],
                        lhsT=vx[kc][:, 2 * p + 1, :],
                        rhs=pts[kc][:, 512:1024],
                        start=(kc == 0),
                        stop=(kc == KC - 1),
                    )
                    yield kc
                key = (p, qb)
                qsl = slice(qb * 512, (qb + 1) * 512)
                for hh, acc in ((0, accA), (1, accB)):
                    asb = asbp.tile([65, 512], F32, tag="asb")
                    nc.vector.tensor_copy(out=asb[:], in_=acc[:])

                    def norm(asb=asb, hh=hh, p=p, qsl=qsl):
                        s0 = small.tile([1, 512], F32, tag="s0")
                        nc.vector.tensor_copy(out=s0[:], in_=asb[64:65, :])
                        rs = small.tile([1, 512], F32, tag="rs")
                        nc.vector.reciprocal_approx_fast(rs[:], s0[:])
                        R = small.tile([64, 512], F32, tag="R")
                        nc.gpsimd.partition_broadcast(R[:], rs[:])
                        if hh == 0:
                            nc.vector.tensor_tensor(
                                out=aT[p][0:64, qsl], in0=asb[0:64, :], in1=R[:],
                                op=MULT,
                            )
                        else:
                            tmpb = small.tile([64, 512], BF, tag="tmpb")
                            nc.vector.tensor_tensor(
                                out=tmpb[:], in0=asb[0:64, :], in1=R[:], op=MULT
                            )
                            nc.sync.dma_start(aT[p][64:P, qsl], tmpb[:])

                    pending_norm.append((key, norm))

            def pv_to(kc):
                g = pv_gen[0]
                if g is None:
                    return
                while v_done[0] <= kc and (vq or reg):
                    if not pump_vq():
                        pump_reg(8)
                try:
                    while next(g) < kc:
                        pass
                except StopIteration:
                    pv_gen[0] = None

            def pv_drain():
                g = pv_gen[0]
                if g is None:
                    return
                try:
                    while True:
                        next(g)
                except StopIteration:
                    pv_gen[0] = None

            def emit_unit(p, qb, rate=2, v_follow=False):
                need_k = 1 if p == 0 else 4
                while not (k_parts.get(p, 0) >= need_k and q_done.get((p, qb))):
                    if not pump_reg(8):
                        raise RuntimeError(f"missing k/q fillers for {(p, qb)}")
                qsl = slice(qb * 512, (qb + 1) * 512)
                pts = [None] * KC
                for kc in range(KC):
                    while k_parts.get(p, 0) <= kc // 4:
                        if not pump_reg(8):
                            raise RuntimeError(f"missing kt chunk for {(p, kc)}")
                    sc = sc_ps.tile([P, 1024], F32, tag="sc")
                    nc.tensor.matmul(
                        sc[:, 0:512],
                        lhsT=kt[p][0:64, kc * P : (kc + 1) * P],
                        rhs=qt[p][0:64, qsl],
                    )
                    nc.tensor.matmul(
                        sc[:, 512:1024],
                        lhsT=kt[p][64:P, kc * P : (kc + 1) * P],
                        rhs=qt[p][64:P, qsl],
                    )
                    pt = ptpool.tile([P, 1024], BF, tag="pt")
                    nc.scalar.activation(out=pt[:], in_=sc[:], func=EXP, scale=SCALE)
                    pts[kc] = pt
                    if v_follow:
                        while v_done[0] <= min(kc + 3, KC - 1):
                            if not pump_vq():
                                break
                    pv_to(kc)
                    pump_reg(1 if (vq or pv_gen[0]) else rate)
                pv_drain()
                pv_gen[0] = gen_unit_pv(p, qb, pts)


            # ---- schedule ----
            # dense pre-attention: K0 seq chunk 0 + Q0 qb0; the rest
            # streams as fillers in dependency order
            for _ in gen_k(0, 0):
                pass
            for _ in gen_q(0, 0):
                pass
            reg.append(gen_k(0, 1))
            reg.append(gen_k(0, 2))
            reg.append(gen_k(0, 3))
            reg.append(gen_q(0, 1))
            for sm in range(KC):
                vq.append(gen_v(sm))
            reg.append(gen_q(0, 2))
            reg.append(gen_q(0, 3))

            for p in range(NPAIR):
                for qb in range(QB):
                    emit_unit(p, qb, v_follow=(p == 0 and qb <= 1))
                    flush_norm()
                    for key in dict.fromkeys(flushed):
                        if key[0] == NPAIR - 1:
                            reg.append(gen_outproj(key[1]))
                    flushed.clear()
                    if p + 1 < NPAIR:
                        if qb == 0:
                            reg.append(gen_k(p + 1, 0))
                            reg.append(gen_k(p + 1, 1))
                        elif qb == 1:
                            reg.append(gen_k(p + 1, 2))
                            reg.append(gen_k(p + 1, 3))
                            reg.append(gen_q(p + 1, 0))
                        elif qb == 2:
                            reg.append(gen_q(p + 1, 1))
                            reg.append(gen_q(p + 1, 2))
                            reg.append(gen_q(p + 1, 3))

            # epilogue: drain last unit's deferred PVs, then final norms
            pv_drain()
            flush_norm()
            for key in dict.fromkeys(flushed):
                if key[0] == NPAIR - 1:
                    reg.append(gen_outproj(key[1]))
            flushed.clear()
            while reg:
                pump_reg(1 << 30)

    nc.finalize()
    _NC = nc
    return nc


def make_in_maps(x, qkv_w, qkv_b, proj_w, proj_b):
    bf16 = ml_dtypes.bfloat16
    x = np.asarray(x, dtype=np.float32)
    qkv_w = np.asarray(qkv_w, dtype=np.float32)
    qkv_b = np.asarray(qkv_b, dtype=np.float32)
    proj_w = np.asarray(proj_w, dtype=np.float32)
    in_maps = []
    for c in range(8):
        b, hh = divmod(c, 2)
        qsl = slice(hh * HDIM, (hh + 1) * HDIM)
        ksl = slice(E + hh * HDIM, E + (hh + 1) * HDIM)
        vsl = slice(2 * E + hh * HDIM, 2 * E + (hh + 1) * HDIM)
        wqkv = np.concatenate([qkv_w[:, ksl], qkv_w[:, qsl], qkv_w[:, vsl]], axis=1)
        in_maps.append(
            {
                "xt": np.ascontiguousarray(x[b].T).astype(bf16),
                "wqkv": np.ascontiguousarray(wqkv).astype(bf16),
                "wp": np.ascontiguousarray(proj_w[qsl, :]).astype(bf16),
                "bq": np.ascontiguousarray(qkv_b[qsl]),
                "bk": np.ascontiguousarray(qkv_b[ksl]),
                "bv_bf": np.ascontiguousarray(qkv_b[vsl]).astype(bf16),
            }
        )
    return in_maps


def assemble_out(results, proj_b):
    out = np.empty((4, SEQ, E), dtype=np.float32)
    pb = np.asarray(proj_b, dtype=np.float32)
    for b in range(4):
        a = np.asarray(results[2 * b]["out"], dtype=np.float32)
        c = np.asarray(results[2 * b + 1]["out"], dtype=np.float32)
        out[b] = a + c + pb
    return out


def run(inputs, trace=False):
    from concourse.bass_utils import run_bass_kernel_spmd

    nc = build_nc()
    in_maps = make_in_maps(**inputs)
    res = run_bass_kernel_spmd(nc, in_maps, core_ids=list(range(8)), trace=trace)
    return assemble_out(res.results, inputs["proj_b"]), res


def kernel(x, qkv_w, qkv_b, proj_w, proj_b):
    out, _ = run(
        dict(x=x, qkv_w=qkv_w, qkv_b=qkv_b, proj_w=proj_w, proj_b=proj_b),
        trace=False,
    )
    return out
